# revision 1
# baseline (speedup 1.0000x reference)
"""Trainium2 Bass kernel for nn_DFVAE (3-stage MoE routing with sorted ids).

Static chunk-grid strategy (N=16384, LD=512, experts (8,6,16), 8 cores,
contiguous row shards, bf16 end-to-end):
  - Per (core, stage): 4 STATIC 512-row chunk windows at rows 512j (expert =
    run covering the chunk start), plus F_s dynamic 256-wide "fix" windows
    that rewrite rows between an unaligned run start and the next chunk
    boundary (<=2 fixes per boundary).  Fix windows may spill past row 2048
    into a 256-row pad of the activation tile (memset once, never stored).
  - Weights are host-packed PER CORE in window-slot order (bf16, lhsT
    layout), so every matmul lhsT is a STATIC SBUF address (PE lhsT cannot
    take register offsets).  Only fix windows use dynamic row offsets
    (values_load on PE/ACT/DVE).
  - Activations bf16 in two ping-pong SBUF tiles (A->B->A->B); static APs
    keep Tile's dependency tracking precise so DMA/compute pipeline.
  - z loaded in 4 per-chunk DMAs (pipelined head); output stored per chunk.
  - PSUM evacuation (bias add + relu) split between ACT (m even) and DVE
    (m odd).
"""
import numpy as np
import ml_dtypes

import concourse.mybir as mybir
import concourse.tile as tile
from concourse import bacc, bass_utils
from concourse.bass import ds

N = 16384
LD = 512
NCORES = 8
SH = N // NCORES      # 2048 rows per core
P = 128
KO = LD // P          # 4 contraction/feature subtiles
CH = 512              # static chunk rows
NCH = SH // CH        # 4 chunks per core
FIXW = 256            # fix window rows
PAD = 256             # activation tile pad rows (fix spill)
STAGE_E = (8, 6, 16)

BF16 = ml_dtypes.bfloat16

LAST_RESULTS = None  # test harness reads exec_time_ns off this

_program_cache = {}


def _core_fixes(loc, hazard=None):
    """Fix list [(start, expert)...] for one core's id vector.

    A single fix spills up to FIXW rows past its run; that is only rewritten
    by a later fix when the following run starts unaligned.  A run that ends
    exactly on a chunk boundary (next run aligned, no fix) would leave the
    spill corrupt -- flag it so the caller can fall back.
    """
    starts = np.flatnonzero(np.diff(loc)) + 1
    fl = []
    for i, bp in enumerate(starts):
        bp = int(bp)
        if bp % CH == 0:
            continue
        run_end = int(starts[i + 1]) if i + 1 < len(starts) else SH
        cover_end = min(run_end, (bp // CH + 1) * CH)
        if cover_end <= bp:
            continue
        e = int(loc[bp])
        ln = cover_end - bp
        if ln <= FIXW:
            fl.append((bp, e))
            if (hazard is not None and ln < FIXW and run_end % CH == 0
                    and run_end < SH):
                hazard.append(bp)
        else:
            fl.append((bp, e))
            fl.append((cover_end - FIXW, e))
    fl.sort()
    return fl


def _assign_rows(ids_all):
    """Order-preserving row->core assignment that steers breakpoint offsets.

    Cores are built from the global stream; core c may defer a tail slice of
    its first atomic block to core c+1 (prepended there).  Removing g rows
    early shifts every later breakpoint left by g (mod 512), turning 2-fix
    boundaries (offset < 256) into 1-fix ones; the deferred rows create one
    junction boundary in the next core, whose offset we also control.
    """
    trip = ids_all[0].astype(np.int64) * 10000 + ids_all[1] * 100 + ids_all[2]
    block_of = np.cumsum(np.diff(trip, prepend=trip[0]) != 0)

    # beam over (donor block, donation size); score = per-stage max fixes
    # across cores (the SPMD shape cost), then total fixes
    beam = [(0, 0, np.empty(0, np.int64), (0, 0, 0), 0, [])]
    for c in range(NCORES):
        nxt_states = []
        for _, cursor, bag, fmax, ftot, rs in beam:
            b = len(bag)
            # candidate donations: (hi, g) = remove rows [hi-g, hi) where
            # [.., hi) is the in-range tail of some atomic block
            cands = {(0, 0)}
            if c < NCORES - 1:
                base_end = cursor + SH - b
                # block portions fully inside the base range
                blks = block_of[cursor:base_end]
                ends = np.flatnonzero(np.diff(blks)) + 1  # block ends (local)
                los = np.concatenate([[0], ends])
                his = np.concatenate([ends, [SH - b]])
                # breakpoint repair targets from the unshifted layout
                loc0 = np.concatenate([bag, np.arange(cursor, base_end)])
                wants = set()
                for s in range(3):
                    loc = ids_all[s][loc0]
                    for bp in (np.flatnonzero(np.diff(loc)) + 1):
                        off = int(bp) % CH
                        for tgt in (0, FIXW, 384):
                            gg = (tgt - off) % CH
                            if gg:
                                wants.add((int(bp), gg))
                for lo, hi in zip(los.tolist(), his.tolist()):
                    avail = hi - lo
                    glo = cursor + hi  # global end of this block portion
                    for bp, gg in wants:
                        # donor at/before the repaired breakpoint
                        if gg < avail and b + hi <= bp + gg:
                            cands.add((glo, gg))
                    if avail > 256:
                        cands.add((glo, 256))
            for hi, g in sorted(cands):
                take = SH - b + g
                if cursor + take > N or (c == NCORES - 1 and
                                         cursor + take != N):
                    continue
                if g:
                    idx = np.concatenate([
                        bag,
                        np.arange(cursor, hi - g),
                        np.arange(hi, cursor + take),
                    ])
                else:
                    idx = np.concatenate([bag,
                                          np.arange(cursor, cursor + take)])
                if len(idx) != SH:
                    continue
                fcs = [len(_core_fixes(ids_all[s][idx])) for s in range(3)]
                nf = (max(fmax[0], fcs[0]), max(fmax[1], fcs[1]),
                      max(fmax[2], fcs[2]))
                key = (nf[0] + nf[1] + nf[2], nf[2], ftot + sum(fcs))
                nbag = np.arange(hi - g, hi) if g else np.empty(0, np.int64)
                nxt_states.append((key, cursor + take, nbag,
                                   nf, ftot + sum(fcs), rs + [idx]))
        nxt_states.sort(key=lambda st: st[0])
        seen = set()
        beam = []
        zero_path = None
        for st in nxt_states:
            sk = (st[1], len(st[2]), int(st[2][0]) if len(st[2]) else -1)
            if st[1] == (c + 1) * SH and len(st[2]) == 0:
                zero_path = zero_path or st
            if sk in seen:
                continue
            seen.add(sk)
            beam.append(st)
            if len(beam) >= 24:
                break
        if zero_path is not None and zero_path not in beam:
            beam.append(zero_path)
    key, cursor, bag, fmax, ftot, rows = beam[0]
    assert cursor == N and len(bag) == 0, (cursor, len(bag))
    if sum(fmax) >= sum(
            max(len(_core_fixes(ids_all[s][c * SH:(c + 1) * SH]))
                for c in range(NCORES)) for s in range(3)):
        return None  # no better than contiguous
    return rows


def _structure(ids_all, rows=None):
    """Per stage: (chunk_experts[8][4], fixes[8]=[(start,expert)...], F)."""
    out = []
    for s in range(3):
        ids = ids_all[s]
        chunk_e = np.zeros((NCORES, NCH), np.int64)
        fixes = []
        for c in range(NCORES):
            loc = ids[rows[c]] if rows is not None else ids[c * SH:(c + 1) * SH]
            for j in range(NCH):
                chunk_e[c, j] = loc[j * CH]
            fixes.append(_core_fixes(loc))
        F = max(len(f) for f in fixes)
        for c in range(NCORES):
            fl = fixes[c]
            filler = fl[-1] if fl else (0, int(chunk_e[c][0]))
            while len(fl) < F:
                fl.append(filler)
        out.append((chunk_e, fixes, F))
    return out


def _pack_w(W):
    """[E, LD, LD] -> [E, P, KO*LD] lhsT layout (k-major blocks)."""
    E = W.shape[0]
    return np.ascontiguousarray(
        W.reshape(E, KO, P, LD).transpose(0, 2, 1, 3).reshape(E, P, KO * LD))


WARMUP = 24
GAPFILL = (0, 0, 0, 0)
EVAC_FLIP = False


def _build_program(F, has_bias=True):
    F0, F1, F2 = F
    S = [NCH + F0, NCH + F1, NCH + F2]
    S_tot = sum(S)
    F_tot = F0 + F1 + F2
    nc = bacc.Bacc("TRN2", target_bir_lowering=False, debug=False,
                   enable_asserts=False, num_devices=NCORES)
    bf = mybir.dt.bfloat16
    f32 = mybir.dt.float32
    i32 = mybir.dt.int32
    PE = mybir.EngineType.PE
    ACT = mybir.EngineType.Activation
    DVE = mybir.EngineType.DVE

    ND = max(F_tot, 1)
    zT = nc.dram_tensor("zT", [LD, SH], bf, kind="ExternalInput").ap()
    Wt = nc.dram_tensor("Wpk", [S_tot * P, KO * LD], bf, kind="ExternalInput").ap()
    Bt = (nc.dram_tensor("bias", [P, S_tot * KO], f32, kind="ExternalInput").ap()
          if has_bias else None)
    Dt = nc.dram_tensor("desc", [1, ND], i32, kind="ExternalInput").ap()
    Ot = nc.dram_tensor("outT", [LD, SH], bf, kind="ExternalOutput").ap()
    # stage-2 fix results land in disjoint static staging; host merges them
    Ft = nc.dram_tensor("fixO", [LD, max(F2, 1) * FIXW], bf,
                        kind="ExternalOutput").ap()

    zv = zT.rearrange("(ko p) r -> p ko r", p=P)
    ov = Ot.rearrange("(ko p) r -> p ko r", p=P)
    fv = Ft.rearrange("(ko p) r -> p ko r", p=P)
    Wv = Wt.rearrange("(s p) c -> s p c", p=P)

    soff = [0, S[0], S[0] + S[1]]
    doff = [0, F0, F0 + F1]

    with tile.TileContext(nc) as tc:
        with (
            tc.tile_pool(name="const", bufs=1) as cpool,
            tc.tile_pool(name="ps512", bufs=5, space="PSUM") as pp5,
            tc.tile_pool(name="ps256", bufs=3, space="PSUM") as pp2,
        ):
            actA = cpool.tile([P, KO, SH + PAD], bf)
            actB = cpool.tile([P, KO, SH + PAD], bf)
            fixout = [cpool.tile([P, KO, FIXW], bf, name=f"fo{f}", tag=f"fo{f}")
                      for f in range(F2)]

            w_sb = []
            for s in range(3):
                row = [cpool.tile([P, KO * LD], bf, name=f"w{s}_{j}", tag=f"w{s}_{j}")
                       for j in range(S[s])]
                w_sb.append(row)

            # pad memsets first: no DMA deps, and the A-pad doubles as the
            # all-zero operand for PE warm-up matmuls during the DMA head
            nc.gpsimd.memset(actA[:, :, SH:SH + PAD], 0.0)
            nc.gpsimd.memset(actB[:, :, SH:SH + PAD], 0.0)

            # head: first chunk's weights and z split by k-block so the k=0
            # matmul can start after ~2 small transfers
            # halves everywhere: 0.73us transfers stay just above the
            # ~0.65us per-DMA issue chain, so the DMA engine never idles
            H = KO * LD // 2  # k01 | k23 halves of a weight slot
            for h in range(2):
                nc.sync.dma_start(w_sb[0][0][:, h * H:(h + 1) * H],
                                  Wv[soff[0]][:, h * H:(h + 1) * H])
                nc.sync.dma_start(actA[:, 2 * h:2 * h + 2, 0:CH],
                                  zv[:, 2 * h:2 * h + 2, 0:CH])
                if h == 0 and has_bias:
                    # only stage-0 chunk biases are needed on the head chain
                    bias_sb = cpool.tile([P, S_tot * KO], f32)
                    nc.sync.dma_start(bias_sb[:, 0:NCH * KO], Bt[:, 0:NCH * KO])
            for j in range(1, NCH):
                nc.sync.dma_start(w_sb[0][j][:, 0:H], Wv[soff[0] + j][:, 0:H])
                nc.sync.dma_start(actA[:, 0:2, j * CH:(j + 1) * CH],
                                  zv[:, 0:2, j * CH:(j + 1) * CH])
                nc.sync.dma_start(w_sb[0][j][:, H:], Wv[soff[0] + j][:, H:])
                nc.sync.dma_start(actA[:, 2:4, j * CH:(j + 1) * CH],
                                  zv[:, 2:4, j * CH:(j + 1) * CH])
            if has_bias:
                nc.sync.dma_start(bias_sb[:, NCH * KO:], Bt[:, NCH * KO:])
            for f in range(F0):
                nc.sync.dma_start(w_sb[0][NCH + f][:, 0:H],
                                  Wv[soff[0] + NCH + f][:, 0:H])
                nc.sync.dma_start(w_sb[0][NCH + f][:, H:],
                                  Wv[soff[0] + NCH + f][:, H:])
            desc_sb = cpool.tile([1, ND], i32)
            nc.sync.dma_start(desc_sb[:], Dt)
            for s in (1, 2):
                for j in range(S[s]):
                    nc.sync.dma_start(w_sb[s][j][:], Wv[soff[s] + j])

            # PE p-state warm-up on the zeroed pad while the head DMAs land
            psw = pp2.tile([P, FIXW], f32, tag="psf", name="psw")

            def dummies(n):
                for i in range(n):
                    nc.tensor.matmul(psw[:, 0:64],
                                     lhsT=actA[:, 0, SH:SH + P],
                                     rhs=actA[:, 0, SH:SH + 64],
                                     start=True, stop=True)

            dummies(WARMUP)

            flip_ctr = [0]

            def evac(m, dst_ap, psum_ap, bias_ap):
                par = m % 2
                if EVAC_FLIP:
                    par = (m + flip_ctr[0]) % 2
                    if m == KO - 1:
                        flip_ctr[0] += 1
                if par == 0:
                    if has_bias:
                        nc.scalar.activation(dst_ap, psum_ap,
                                             mybir.ActivationFunctionType.Relu,
                                             bias=bias_ap)
                    else:
                        nc.scalar.activation(dst_ap, psum_ap,
                                             mybir.ActivationFunctionType.Relu)
                else:
                    nc.vector.tensor_scalar(dst_ap, psum_ap,
                                            bias_ap if has_bias else 0.0, 0.0,
                                            mybir.AluOpType.add,
                                            mybir.AluOpType.max)

            def chunk_window(s, cur, nxt, j, lo=None, w=None):
                wt = w_sb[s][j]
                lo = j * CH if lo is None else lo
                w = CH if w is None else w
                for m in range(KO):
                    if w == CH:
                        psum = pp5.tile([P, CH], f32, tag="ps", name="psum")
                    else:
                        psum = pp2.tile([P, FIXW], f32, tag="psf", name="psumh")
                    for k in range(KO):
                        nc.tensor.matmul(
                            psum[:],
                            lhsT=wt[:, k * LD + m * P: k * LD + (m + 1) * P],
                            rhs=cur[:, k, lo:lo + w],
                            start=(k == 0), stop=(k == KO - 1))
                    bias_ap = (bias_sb[:, (soff[s] + j) * KO + m:
                                       (soff[s] + j) * KO + m + 1]
                               if has_bias else None)
                    evac(m, nxt[:, m, lo:lo + w], psum[:], bias_ap)
                if s == 2:
                    # chunk rows are pre-fix; fixed rows are merged on host
                    # from the fixO staging written by the stage-2 fixes
                    nc.sync.dma_start(ov[:, :, j * CH:(j + 1) * CH],
                                      nxt[:, :, j * CH:(j + 1) * CH])

            def fix_window(s, cur, nxt, f):
                idx = doff[s] + f
                last = s == 2
                r = nc.values_load(
                    desc_sb[0:1, idx:idx + 1],
                    engines=[PE] if last else [PE, ACT, DVE],
                    min_val=0, max_val=SH - 1,
                    skip_runtime_bounds_check=True)
                wt = w_sb[s][NCH + f]
                fo = fixout[f] if last else None
                for m in range(KO):
                    psum = pp2.tile([P, FIXW], f32, tag="psf", name="psumf")
                    for k in range(KO):
                        nc.tensor.matmul(
                            psum[:],
                            lhsT=wt[:, k * LD + m * P: k * LD + (m + 1) * P],
                            rhs=cur[:, k, ds(r, FIXW)],
                            start=(k == 0), stop=(k == KO - 1))
                    bias_ap = (bias_sb[:, (soff[s] + NCH + f) * KO + m:
                                       (soff[s] + NCH + f) * KO + m + 1]
                               if has_bias else None)
                    if last:
                        # store each ko-half as soon as its evacs land:
                        # shortens the final evac->store->semaphore tail
                        evac(m, fo[:, m, :], psum[:], bias_ap)
                        if m == 1:
                            nc.sync.dma_start(
                                fv[:, 0:2, f * FIXW:(f + 1) * FIXW],
                                fo[:, 0:2, :])
                        elif m == KO - 1:
                            nc.sync.dma_start(
                                fv[:, 2:4, f * FIXW:(f + 1) * FIXW],
                                fo[:, 2:4, :])
                    else:
                        evac(m, nxt[:, m, ds(r, FIXW)], psum[:], bias_ap)

            for s in range(3):
                cur, nxt = (actA, actB) if s % 2 == 0 else (actB, actA)
                for j in range(NCH):
                    chunk_window(s, cur, nxt, j)
                    if s == 0:
                        # keep the PE p-state ramp pinned across head DMA gaps
                        dummies(GAPFILL[j])
                for f in range(F[s]):
                    fix_window(s, cur, nxt, f)
    nc.compile()
    return nc


def _prep_inputs(z, Ws, bs, stages, rows):
    F = [st[2] for st in stages]
    S = [NCH + f for f in F]
    S_tot = sum(S)
    F_tot = sum(F)
    Wpk_s = [_pack_w(Ws[s]).astype(BF16) for s in range(3)]
    z32 = np.asarray(z, np.float32)

    in_maps = []
    for c in range(NCORES):
        zc = z32[rows[c]] if rows is not None else z32[c * SH:(c + 1) * SH]
        zTc = np.ascontiguousarray(zc.T).astype(BF16)
        slots = []   # (stage, expert)
        for s in range(3):
            chunk_e, fixes, _ = stages[s]
            for j in range(NCH):
                slots.append((s, int(chunk_e[c][j])))
            for r, e in fixes[c]:
                slots.append((s, int(e)))
        Wpk = np.empty((S_tot * P, KO * LD), BF16)
        bias = np.empty((P, S_tot * KO), np.float32)
        for i, (s, e) in enumerate(slots):
            Wpk[i * P:(i + 1) * P] = Wpk_s[s][e]
            bias[:, i * KO:(i + 1) * KO] = bs[s][e].reshape(KO, P).T
        desc = np.zeros((1, max(F_tot, 1)), np.int32)
        i = 0
        for s in range(3):
            _, fixes, _ = stages[s]
            for r, e in fixes[c]:
                desc[0, i] = r
                i += 1
        in_maps.append({
            "zT": zTc,
            "Wpk": Wpk,
            "bias": bias,
            "desc": desc,
        })
    return in_maps


def _kernel_numpy_fallback(z, Ws, bs, ids_all):
    out = np.asarray(z, np.float32)
    for s in range(3):
        nxt = np.empty_like(out)
        ids = ids_all[s]
        for e in range(Ws[s].shape[0]):
            mask = ids == e
            if mask.any():
                nxt[mask] = np.maximum(out[mask] @ Ws[s][e] + bs[s][e], 0.0)
        out = nxt
    return out


def kernel(z, W_dataset, b_dataset, W_assay, b_assay, W_donor, b_donor,
           dataset_ids, assay_ids, donor_ids):
    global LAST_RESULTS
    ids_all = [
        np.asarray(dataset_ids, np.int32),
        np.asarray(assay_ids, np.int32),
        np.asarray(donor_ids, np.int32),
    ]
    Ws = [
        np.ascontiguousarray(np.asarray(W_dataset, np.float32)),
        np.ascontiguousarray(np.asarray(W_assay, np.float32)),
        np.ascontiguousarray(np.asarray(W_donor, np.float32)),
    ]
    bs = [
        np.asarray(b_dataset, np.float32),
        np.asarray(b_assay, np.float32),
        np.asarray(b_donor, np.float32),
    ]
    if any(np.any(np.diff(ids) < 0) for ids in ids_all):
        return _kernel_numpy_fallback(z, Ws, bs, ids_all)

    import os
    rows = None
    if os.environ.get("MOE_ASSIGN", "1") == "1":
        try:
            rows = _assign_rows(ids_all)
        except AssertionError:
            rows = None
    hazard = []
    for s in range(3):
        for c in range(NCORES):
            loc = ids_all[s][rows[c]] if rows is not None \
                else ids_all[s][c * SH:(c + 1) * SH]
            _core_fixes(loc, hazard)
    if hazard:
        return _kernel_numpy_fallback(z, Ws, bs, ids_all)
    stages = _structure(ids_all, rows)
    F = tuple(st[2] for st in stages)
    has_bias = any(np.any(b) for b in bs)
    key = (F, has_bias)
    if key not in _program_cache:
        _program_cache[key] = _build_program(F, has_bias)
    nc = _program_cache[key]
    in_maps = _prep_inputs(z, Ws, bs, stages, rows)
    if not has_bias:
        for m in in_maps:
            m.pop("bias", None)
    res = bass_utils.run_bass_kernel_spmd(nc, in_maps, core_ids=list(range(NCORES)))
    LAST_RESULTS = res

    out = np.empty((N, LD), np.float32)
    for c in range(NCORES):
        blk = res.results[c]["outT"].astype(np.float32).T  # [SH, LD] pre-fix
        fo = res.results[c]["fixO"].astype(np.float32).T   # [F2*FIXW, LD]
        for f, (r, e) in enumerate(stages[2][1][c]):       # ASC order merge
            n = min(FIXW, SH - r)
            blk[r:r + n] = fo[f * FIXW:f * FIXW + n]
        if rows is not None:
            out[rows[c]] = blk
        else:
            out[c * SH:(c + 1) * SH] = blk
    return out



# revision 55
# speedup vs baseline: 1.1228x; 1.1228x over previous
"""Trainium2 Bass kernel for nn_DFVAE (3-stage MoE routing with sorted ids).

Static chunk-grid strategy (N=16384, LD=512, experts (8,6,16), 8 cores,
bf16 end-to-end), v2:
  - Per (core, stage): 4 STATIC 512-row chunk windows (expert = id at the
    chunk start) plus F_s dynamic fix windows with PER-SLOT STATIC WIDTHS
    (program signature) that rewrite rows between an unaligned run start and
    the next chunk boundary.  Unused fix slots park at row SH (the zeroed
    pad), writing zeros to pad rows only.
  - Weights host-packed PER CORE in window-slot order (bf16 lhsT layout);
    matmul lhsT addresses are static.  Fix windows use values_load row
    offsets from a desc vector.
  - Activations bf16 in two ping-pong SBUF tiles; stage-2 chunk outputs
    stored in k01/k23 halves as soon as their evacs land; stage-2 fixes
    (ordered widest..narrowest) write disjoint fixO staging merged on host.
  - Head: chunk-0 weights+z arrive in k-quarters and chunk 0 runs a k-outer
    loop so the first matmul needs only 2 small DMAs; PE p-state ramp is
    held by warm-up matmuls on a tiny zeroed tile.
"""
import numpy as np
import ml_dtypes

import concourse.mybir as mybir
import concourse.tile as tile
from concourse import bacc, bass_utils
from concourse.bass import ds

N = 16384
LD = 512
NCORES = 8
SH = N // NCORES      # 2048 rows per core
P = 128
KO = LD // P          # 4 contraction/feature subtiles
CH = 512              # static chunk rows
NCH = SH // CH        # 4 chunks per core
PAD = 512             # activation tile pad rows (fix spill + filler park)
STAGE_E = (8, 6, 16)

BF16 = ml_dtypes.bfloat16

LAST_RESULTS = None  # test harness reads exec_time_ns off this

_program_cache = {}

WARMUP = 63
GAPFILL = (0, 0, 0, 0)


def _stage_fixes(loc):
    """Per-breakpoint fix options for one core's id vector.

    Each chunk's expert is the value of its LONGEST run, so every other run
    inside the chunk needs one fix window.  A fix for run [a, b) may be
    anchored at its end (r = b - W, W >= b - a) because back-spill rewrites
    same-run rows of the previous chunk with identical values, or at its
    start when the run crosses into the next chunk (forward spill rewrites
    same-run rows the next chunk also computes).

    Returns (chunk_e[NCH], fixes=[(anchor, expert, need, maxw)...]) where
    anchor is 'r' (r = bp, forward) or 'l' (r = end - W, backward); fixes
    sorted by position.
    """
    starts = np.flatnonzero(np.diff(loc)) + 1
    bounds = np.concatenate([[0], starts, [SH]]).astype(np.int64)
    chunk_e = []
    fixes = []
    for j in range(NCH):
        c0, c1 = j * CH, (j + 1) * CH
        # runs intersecting this chunk, clipped
        runs = []
        for i in range(len(bounds) - 1):
            a, b = int(bounds[i]), int(bounds[i + 1])
            if b <= c0 or a >= c1:
                continue
            runs.append((max(a, c0), min(b, c1), int(loc[max(a, c0)]),
                         a, b))  # clipped start/end, value, full start/end
        best_j = max(range(len(runs)),
                     key=lambda i: runs[i][1] - runs[i][0])
        chunk_e.append(runs[best_j][2])
        for i, (a, b, v, fa, fb) in enumerate(runs):
            if i == best_j:
                continue
            need = b - a
            if fb > c1:
                # run crosses chunk end: forward-anchored, may spill right
                fixes.append(('r', a, v, need, min(fb - a, CH)))
            else:
                # backward-anchored at run (clipped) end, may spill left
                fixes.append(('l', b, v, need, min(b - fa, CH)))
    fixes.sort(key=lambda f: f[1])
    return chunk_e, fixes


def slot_cost(slots):
    # a fix window costs ~2.1us of PE-SEQ dispatch (partially hidden under
    # chunk slack) and 6.67ns/row of PE-engine time
    return sum(max(2100.0, 6.67 * w + 400.0) for w in slots)


def _match_core(fixes, slots):
    """Injective fix->slot matching with need <= W <= maxw; None if none.

    Slots sorted desc; prefer using wider slots for wider needs.  F is tiny
    (<=5) so brute-force permutations are fine.
    """
    import itertools
    F = len(slots)
    n = len(fixes)
    order = sorted(range(n), key=lambda i: -fixes[i][3])
    for perm in itertools.permutations(range(F), n):
        ok = True
        for i, k in zip(order, perm):
            _, _, _, need, maxw = fixes[i]
            if not (need <= slots[k] <= maxw):
                ok = False
                break
        if ok:
            asg = [None] * F
            for i, k in zip(order, perm):
                anchor, pos, v, need, maxw = fixes[i]
                W = slots[k]
                r = pos if anchor == 'r' else pos - W
                asg[k] = (r, v, W)
            for k in range(F):
                if asg[k] is None:
                    asg[k] = (SH, 0, slots[k])
            return asg
    return None


def _plan_stage(per_core):
    """Slot widths for one stage with spill-allowance-aware matching.

    Returns (slots, assign) or None.  assign[c] = [(r, expert, W)...] in
    slot order with fillers (r=SH, W) for unused slots.
    """
    lists = [sorted(fixes, key=lambda f: -f[3]) for _, fixes in per_core]
    F = max((len(fx) for fx in lists), default=0)
    base = [0] * F
    for fx in lists:
        for k, f in enumerate(fx):
            base[k] = max(base[k], f[3])
    # candidate slot multisets: rank-max needs, then exact per-core widths
    cands = [base]
    widths = sorted({f[3] for fx in lists for f in fx}, reverse=True)
    if widths and len(widths) <= F + 2:
        from collections import Counter
        mult = Counter()
        for fx in lists:
            cc = Counter(f[3] for f in fx)
            for w, n in cc.items():
                mult[w] = max(mult[w], n)
        exact = sorted(mult.elements(), reverse=True)
        if len(exact) <= F + 2:
            cands.append(exact)
    for slots in cands:
        assign = []
        ok = True
        for fx in lists:
            asg = _match_core(fx, slots)
            if asg is None:
                ok = False
                break
            assign.append(asg)
        if ok:
            return slots, assign
    return None


def _validate(ids_all, rows, plans):
    """Simulate chunk+fix writes in device window order; True iff ids
    reproduced.  Stages 0/1 write in place: chunk j, then fixes placed
    after chunk j (slot order).  Stage 2 fixes go to staging and the host
    merges them after all chunks, i.e. same ordering semantics."""
    for s in range(3):
        plan = plans[s]
        after = plan["after"]
        for c in range(NCORES):
            loc = ids_all[s][rows[c]] if rows is not None \
                else ids_all[s][c * SH:(c + 1) * SH]
            arr = np.empty(SH + PAD, np.int64)
            arr[:] = -1
            for j in range(NCH):
                arr[j * CH:(j + 1) * CH] = plan["chunk_e"][c][j]
                for k, (r, e, w) in enumerate(plan["assign"][c]):
                    if after[k] == j:
                        arr[r:r + w] = e
            if not np.array_equal(arr[:SH], loc):
                return False
    return True


def _make_plans(ids_all, rows):
    plans = []
    for s in range(3):
        per_core = []
        for c in range(NCORES):
            loc = ids_all[s][rows[c]] if rows is not None \
                else ids_all[s][c * SH:(c + 1) * SH]
            per_core.append(_stage_fixes(loc))
        ps = _plan_stage(per_core)
        if ps is None:
            return None
        slots, assign = ps
        # earliest safe interleave point per slot: after the last chunk any
        # core's matched fix touches (incl. spill); stage 0 fixes read z at
        # dynamic offsets so they stay at stage end regardless
        F = len(slots)
        after = []
        for k in range(F):
            aj = 0
            for c in range(NCORES):
                r, e, w = assign[c][k]
                if r < SH:
                    aj = max(aj, (r + w - 1) // CH)
            after.append(min(aj, NCH - 1) if s > 0 else NCH - 1)
        plans.append({
            "slots": slots,
            "chunk_e": [pc[0] for pc in per_core],
            "assign": assign,
            "after": after,
        })
    return plans


def _construct_rows(ids_all):
    """Quantum dealing: 32 contiguous 512-row quanta, grouped so every
    breakpoint's quantum is adjacent (in its core) to the neighbor quantum
    its fix window spills into, then packed into cores balancing fix load.

    Returns rows[8] or None.
    """
    NQ = N // CH
    bps = []
    for s in range(3):
        for bp in (np.flatnonzero(np.diff(ids_all[s])) + 1).tolist():
            bps.append((int(bp), s))
    # union quanta that must stay adjacent (in global order)
    parent = list(range(NQ))

    def find(a):
        while parent[a] != a:
            parent[a] = parent[parent[a]]
            a = parent[a]
        return a

    def union(a, b):
        ra, rb = find(a), find(b)
        if ra != rb:
            parent[max(ra, rb)] = min(ra, rb)

    for pos, s in bps:
        q, x = pos // CH, pos % CH
        if x == 0:
            continue
        if x <= CH // 2 and q >= 1:
            union(q - 1, q)
        elif x > CH // 2 and q + 1 < NQ:
            union(q, q + 1)
    groups = {}
    for q in range(NQ):
        groups.setdefault(find(q), []).append(q)
    pieces = sorted(groups.values(), key=len, reverse=True)
    if pieces and len(pieces[0]) > NCH:
        return None
    # fix load per piece, per stage: sorted-desc width lists
    def piece_load(p):
        lo, hi = p[0] * CH, (p[-1] + 1) * CH
        load = [[], [], []]
        for pos, s in bps:
            if lo <= pos < hi and pos % CH:
                x = pos % CH
                load[s].append(min(x, CH - x))
        for s in range(3):
            load[s].sort(reverse=True)
        return load

    loads = [piece_load(p) for p in pieces]
    cores = [[] for _ in range(NCORES)]
    core_load = [[[], [], []] for _ in range(NCORES)]

    def inc_cost(cl, pl):
        inc = 0.0
        tgt = (1, 1, 2)  # window-count targets per stage
        for s in range(3):
            cur = sorted(cl[s], reverse=True)
            new = sorted(cl[s] + pl[s], reverse=True)
            inc += (sum(max(2100.0, 6.67 * w) for w in new)
                    - sum(max(2100.0, 6.67 * w) for w in cur))
            inc += 4000.0 * max(0, len(new) - tgt[s])
        return inc

    order = sorted(range(len(pieces)),
                   key=lambda i: (-len(pieces[i]),
                                  -sum(sum(l) for l in loads[i])))
    for i in order:
        best_c, best = None, None
        for c in range(NCORES):
            if sum(len(p) for p in cores[c]) + len(pieces[i]) > NCH:
                continue
            v = inc_cost(core_load[c], loads[i])
            if best is None or v < best:
                best_c, best = c, v
        if best_c is None:
            return None
        cores[best_c].append(pieces[i])
        for s in range(3):
            core_load[best_c][s].extend(loads[i][s])
    rows = []
    for c in range(NCORES):
        # boundary-carrying pieces first: their fix windows then sit in low
        # chunks and can interleave early between chunk windows
        def piece_fixes(p):
            lo, hi = p[0] * CH, (p[-1] + 1) * CH
            return -sum(1 for pos, s in bps if lo <= pos < hi and pos % CH)
        idx = []
        for p in sorted(cores[c], key=piece_fixes):
            idx.append(np.arange(p[0] * CH, (p[-1] + 1) * CH,
                                 dtype=np.int64))
        rows.append(np.concatenate(idx))
    return rows


def _assign_rows(ids_all):
    """Order-preserving row->core assignment that steers breakpoint offsets.

    Beam over per-core donations: core c may defer a tail slice of an atomic
    block to core c+1.  Scores by total fix width (exact widths) with
    per-stage max-across-cores structure.
    """
    trip = ids_all[0].astype(np.int64) * 10000 + ids_all[1] * 100 + ids_all[2]
    block_of = np.cumsum(np.diff(trip, prepend=trip[0]) != 0)

    def fix_needs(loc):
        """Sorted-desc fix widths for one core/stage."""
        _, fixes = _stage_fixes(loc)
        return sorted((f[3] for f in fixes), reverse=True)

    def merge_slots(slots, needs):
        out = list(slots) + [0] * max(0, len(needs) - len(slots))
        for k, w in enumerate(needs):
            out[k] = max(out[k], w)
        return out

    beam = [(0, 0, np.empty(0, np.int64), [[], [], []], [])]
    for c in range(NCORES):
        nxt_states = []
        for _, cursor, bag, fmax, rs in beam:
            b = len(bag)
            cands = {(0, 0)}
            if c < NCORES - 1:
                base_end = cursor + SH - b
                blks = block_of[cursor:base_end]
                ends = np.flatnonzero(np.diff(blks)) + 1
                los = np.concatenate([[0], ends])
                his = np.concatenate([ends, [SH - b]])
                loc0 = np.concatenate([bag, np.arange(cursor, base_end)])
                wants = set()
                for s in range(3):
                    loc = ids_all[s][loc0]
                    for bp in (np.flatnonzero(np.diff(loc)) + 1):
                        off = int(bp) % CH
                        for tgt in (0, 128, 192, 256, 320, 384, 416, 448,
                                    480, 496):
                            gg = (tgt - off) % CH
                            if gg:
                                wants.add((int(bp), gg))
                for lo, hi in zip(los.tolist(), his.tolist()):
                    avail = hi - lo
                    glo = cursor + hi
                    for bp, gg in wants:
                        if gg < avail and b + hi <= bp + gg:
                            cands.add((glo, gg))
                    for gfix in (128, 256, 384):
                        if avail > gfix:
                            cands.add((glo, gfix))
            for hi, g in sorted(cands):
                take = SH - b + g
                if cursor + take > N or (c == NCORES - 1 and
                                         cursor + take != N):
                    continue
                if g:
                    idx = np.concatenate([
                        bag,
                        np.arange(cursor, hi - g),
                        np.arange(hi, cursor + take),
                    ])
                else:
                    idx = np.concatenate([bag,
                                          np.arange(cursor, cursor + take)])
                if len(idx) != SH:
                    continue
                nf = [merge_slots(fmax[s], fix_needs(ids_all[s][idx]))
                      for s in range(3)]
                key = (sum(slot_cost(x) for x in nf), )
                nbag = np.arange(hi - g, hi) if g else np.empty(0, np.int64)
                nxt_states.append((key, cursor + take, nbag, nf, rs + [idx]))
        nxt_states.sort(key=lambda st: st[0])
        seen = set()
        beam = []
        for st in nxt_states:
            sk = (st[1], len(st[2]), int(st[2][0]) if len(st[2]) else -1)
            if sk in seen:
                continue
            seen.add(sk)
            beam.append(st)
            if len(beam) >= 48:
                break
    key, cursor, bag, fmax, rows = beam[0]
    assert cursor == N and len(bag) == 0, (cursor, len(bag))
    return rows


def _pack_w(W):
    """[E, LD, LD] -> [E, P, KO*LD] lhsT layout (k-major blocks)."""
    E = W.shape[0]
    return np.ascontiguousarray(
        W.reshape(E, KO, P, LD).transpose(0, 2, 1, 3).reshape(E, P, KO * LD))


def _build_program(F, has_bias=True):
    # F[s] = tuple of (width, after_chunk) per fix slot
    AF = [[a for _, a in F[s]] for s in range(3)]
    W0, W1, W2 = [tuple(w for w, _ in F[s]) for s in range(3)]
    Fs = [len(W0), len(W1), len(W2)]
    S = [NCH + Fs[s] for s in range(3)]
    S_tot = sum(S)
    F_tot = sum(Fs)
    FMAX = max([64] + list(W0) + list(W1) + list(W2))
    W2_off = np.concatenate([[0], np.cumsum(W2)]).astype(int) if Fs[2] \
        else np.zeros(1, int)
    W2_tot = int(W2_off[-1]) if Fs[2] else 0

    nc = bacc.Bacc("TRN2", target_bir_lowering=False, debug=False,
                   enable_asserts=False, num_devices=NCORES)
    bf = mybir.dt.bfloat16
    f32 = mybir.dt.float32
    i32 = mybir.dt.int32
    PE = mybir.EngineType.PE
    ACT = mybir.EngineType.Activation
    DVE = mybir.EngineType.DVE

    ND = max(F_tot, 1)
    zT = nc.dram_tensor("zT", [LD, SH], bf, kind="ExternalInput").ap()
    Wt = nc.dram_tensor("Wpk", [S_tot * P, KO * LD], bf,
                        kind="ExternalInput").ap()
    Bt = (nc.dram_tensor("bias", [P, S_tot * KO], f32,
                         kind="ExternalInput").ap() if has_bias else None)
    Dt = nc.dram_tensor("desc", [1, ND], i32, kind="ExternalInput").ap()
    Ot = nc.dram_tensor("outT", [LD, SH], bf, kind="ExternalOutput").ap()
    Ft = nc.dram_tensor("fixO", [LD, max(W2_tot, 64)], bf,
                        kind="ExternalOutput").ap()

    zv = zT.rearrange("(ko p) r -> p ko r", p=P)
    ov = Ot.rearrange("(ko p) r -> p ko r", p=P)
    fv = Ft.rearrange("(ko p) r -> p ko r", p=P)
    Wv = Wt.rearrange("(s p) c -> s p c", p=P)

    soff = [0, S[0], S[0] + S[1]]
    doff = [0, Fs[0], Fs[0] + Fs[1]]
    H = KO * LD // 2   # half of a weight slot's columns
    Q = KO * LD // 4   # quarter (one k block)

    with tile.TileContext(nc) as tc:
        with (
            tc.tile_pool(name="const", bufs=1) as cpool,
            tc.tile_pool(name="ps512", bufs=5, space="PSUM") as pp5,
            tc.tile_pool(name="psfix", bufs=3, space="PSUM") as ppf,
        ):
            actA = cpool.tile([P, KO, SH + PAD], bf)
            actB = cpool.tile([P, KO, SH + PAD], bf)
            zdum = cpool.tile([P, P + 64], bf)
            fixout = [cpool.tile([P, KO, W2[f]], bf, name=f"fo{f}",
                                 tag=f"fo{f}") for f in range(Fs[2])]

            w_sb = []
            for s in range(3):
                row = [cpool.tile([P, KO * LD], bf, name=f"w{s}_{j}",
                                  tag=f"w{s}_{j}") for j in range(S[s])]
                w_sb.append(row)

            # zdum first so warm-up matmuls start as early as possible
            nc.gpsimd.memset(zdum[:], 0.0)
            nc.gpsimd.memset(actA[:, :, SH:SH + PAD], 0.0)
            nc.gpsimd.memset(actB[:, :, SH:SH + PAD], 0.0)

            # head: chunk-0 weights+z arrive as per-k quarter pairs so the
            # k-outer chunk-0 loop is never starved; chunk 1 in k01/k23
            # halves; chunks 2-3 whole; desc + stage-0 fix weights early so
            # interleaved fix windows aren't blocked
            for j in range(2):
                for h in range(2):
                    nc.sync.dma_start(
                        w_sb[0][j][:, h * H:(h + 1) * H],
                        Wv[soff[0] + j][:, h * H:(h + 1) * H])
                    nc.sync.dma_start(
                        actA[:, 2 * h:2 * h + 2, j * CH:(j + 1) * CH],
                        zv[:, 2 * h:2 * h + 2, j * CH:(j + 1) * CH])
            if has_bias:
                bias_sb = cpool.tile([P, S_tot * KO], f32)
                nc.sync.dma_start(bias_sb[:, 0:NCH * KO], Bt[:, 0:NCH * KO])
            for j in range(2, NCH):
                nc.sync.dma_start(w_sb[0][j][:], Wv[soff[0] + j])
                nc.sync.dma_start(actA[:, :, j * CH:(j + 1) * CH],
                                  zv[:, :, j * CH:(j + 1) * CH])
            if has_bias:
                nc.sync.dma_start(bias_sb[:, NCH * KO:], Bt[:, NCH * KO:])
            desc_sb = cpool.tile([1, ND], i32)
            nc.sync.dma_start(desc_sb[:], Dt)
            for f in range(Fs[0]):
                nc.sync.dma_start(w_sb[0][NCH + f][:], Wv[soff[0] + NCH + f])
            for s in (1, 2):
                for j in range(S[s]):
                    nc.sync.dma_start(w_sb[s][j][:], Wv[soff[s] + j])

            # PE p-state warm-up on the small zeroed tile
            psw = ppf.tile([P, FMAX], f32, tag="psf", name="psw")

            def dummies(n):
                for i in range(n):
                    nc.tensor.matmul(psw[:, 0:64],
                                     lhsT=zdum[:, 0:P],
                                     rhs=zdum[:, P:P + 64],
                                     start=True, stop=True)

            dummies(WARMUP)

            def evac(m, dst_ap, psum_ap, bias_ap):
                if m % 2 == 0:
                    if has_bias:
                        nc.scalar.activation(dst_ap, psum_ap,
                                             mybir.ActivationFunctionType.Relu,
                                             bias=bias_ap)
                    else:
                        nc.scalar.activation(dst_ap, psum_ap,
                                             mybir.ActivationFunctionType.Relu)
                else:
                    nc.vector.tensor_scalar(dst_ap, psum_ap,
                                            bias_ap if has_bias else 0.0, 0.0,
                                            mybir.AluOpType.add,
                                            mybir.AluOpType.max)

            def bias_ap_of(s, slot, m):
                if not has_bias:
                    return None
                col = (soff[s] + slot) * KO + m
                return bias_sb[:, col:col + 1]

            def chunk_koru(s, cur, nxt, j):
                """k-outer chunk: k01 matmuls start as soon as the first
                half-DMA pair lands."""
                wt = w_sb[s][j]
                lo = j * CH
                psums = [pp5.tile([P, CH], f32, tag="ps", name="psum")
                         for _ in range(KO)]
                for k in range(KO):
                    for m in range(KO):
                        nc.tensor.matmul(
                            psums[m][:],
                            lhsT=wt[:, k * LD + m * P: k * LD + (m + 1) * P],
                            rhs=cur[:, k, lo:lo + CH],
                            start=(k == 0), stop=(k == KO - 1))
                for m in range(KO):
                    evac(m, nxt[:, m, lo:lo + CH], psums[m][:],
                         bias_ap_of(s, j, m))

            def chunk_window(s, cur, nxt, j):
                wt = w_sb[s][j]
                lo = j * CH
                last_chunk = s == 2 and j == NCH - 1
                for m in range(KO):
                    psum = pp5.tile([P, CH], f32, tag="ps", name="psum")
                    for k in range(KO):
                        nc.tensor.matmul(
                            psum[:],
                            lhsT=wt[:, k * LD + m * P: k * LD + (m + 1) * P],
                            rhs=cur[:, k, lo:lo + CH],
                            start=(k == 0), stop=(k == KO - 1))
                    if last_chunk and m == KO - 1:
                        # kernel-final evac: split across ACT and DVE so the
                        # final store's wait fires ~300ns earlier
                        hc = CH // 2
                        if has_bias:
                            bap = bias_ap_of(s, j, m)
                            nc.scalar.activation(
                                nxt[:, m, lo:lo + hc], psum[:, 0:hc],
                                mybir.ActivationFunctionType.Relu, bias=bap)
                            nc.vector.tensor_scalar(
                                nxt[:, m, lo + hc:lo + CH], psum[:, hc:CH],
                                bap, 0.0, mybir.AluOpType.add,
                                mybir.AluOpType.max)
                        else:
                            nc.scalar.activation(
                                nxt[:, m, lo:lo + hc], psum[:, 0:hc],
                                mybir.ActivationFunctionType.Relu)
                            nc.vector.tensor_scalar(
                                nxt[:, m, lo + hc:lo + CH], psum[:, hc:CH],
                                0.0, 0.0, mybir.AluOpType.add,
                                mybir.AluOpType.max)
                    else:
                        evac(m, nxt[:, m, lo:lo + CH], psum[:],
                             bias_ap_of(s, j, m))
                    if s == 2 and m == 1:
                        nc.sync.dma_start(ov[:, 0:2, lo:lo + CH],
                                          nxt[:, 0:2, lo:lo + CH])
                    elif last_chunk and m == 2:
                        nc.sync.dma_start(ov[:, 2:3, lo:lo + CH],
                                          nxt[:, 2:3, lo:lo + CH])
                    elif last_chunk and m == KO - 1:
                        nc.sync.dma_start(ov[:, 3:4, lo:lo + CH],
                                          nxt[:, 3:4, lo:lo + CH])
                    elif s == 2 and m == KO - 1:
                        nc.sync.dma_start(ov[:, 2:4, lo:lo + CH],
                                          nxt[:, 2:4, lo:lo + CH])

            def fix_window(s, cur, nxt, f, W):
                idx = doff[s] + f
                last = s == 2
                r = nc.values_load(
                    desc_sb[0:1, idx:idx + 1],
                    engines=[PE] if last else [PE, ACT, DVE],
                    min_val=0, max_val=SH,
                    skip_runtime_bounds_check=True)
                wt = w_sb[s][NCH + f]
                fo = fixout[f] if last else None
                final = last and f == Fs[2] - 1
                for m in range(KO):
                    # the kernel-final window draws psums from the 5-deep
                    # chunk pool so its m3 matmuls never wait on evacs
                    if final:
                        psum = pp5.tile([P, CH], f32, tag="ps", name="psum")
                    else:
                        psum = ppf.tile([P, FMAX], f32, tag="psf",
                                        name="psumf")
                    for k in range(KO):
                        nc.tensor.matmul(
                            psum[:, 0:W],
                            lhsT=wt[:, k * LD + m * P: k * LD + (m + 1) * P],
                            rhs=cur[:, k, ds(r, W)],
                            start=(k == 0), stop=(k == KO - 1))
                    bap = bias_ap_of(s, NCH + f, m)
                    if last:
                        evac(m, fo[:, m, :], psum[:, 0:W], bap)
                        if f == Fs[2] - 1:
                            # kernel-final window: one store, issued once
                            if m == KO - 1:
                                nc.sync.dma_start(
                                    fv[:, :, W2_off[f]:W2_off[f + 1]],
                                    fo[:])
                        elif m == 1:
                            nc.sync.dma_start(
                                fv[:, 0:2, W2_off[f]:W2_off[f + 1]],
                                fo[:, 0:2, :])
                        elif m == KO - 1:
                            nc.sync.dma_start(
                                fv[:, 2:4, W2_off[f]:W2_off[f + 1]],
                                fo[:, 2:4, :])
                    else:
                        evac(m, nxt[:, m, ds(r, W)], psum[:, 0:W], bap)

            for s in range(3):
                cur, nxt = (actA, actB) if s % 2 == 0 else (actB, actA)
                Ws = (W0, W1, W2)[s]
                for j in range(NCH):
                    if s == 0 and j < 2:
                        chunk_koru(s, cur, nxt, j)
                        dummies(GAPFILL[j])
                    else:
                        chunk_window(s, cur, nxt, j)
                        if s == 0:
                            dummies(GAPFILL[j])
                    # fix slot k runs right after the last chunk any core's
                    # matched fix touches (stage-0 fixes pinned to stage end)
                    for f in range(Fs[s]):
                        if AF[s][f] == j:
                            fix_window(s, cur, nxt, f, Ws[f])
    nc.compile()
    return nc


def _prep_inputs(z, Ws, bs, plans, rows):
    Fs = [len(plans[s]["slots"]) for s in range(3)]
    S = [NCH + f for f in Fs]
    S_tot = sum(S)
    F_tot = sum(Fs)
    Wpk_s = [_pack_w(Ws[s]).astype(BF16) for s in range(3)]
    z32 = np.asarray(z, np.float32)

    in_maps = []
    for c in range(NCORES):
        zc = z32[rows[c]] if rows is not None else z32[c * SH:(c + 1) * SH]
        zTc = np.ascontiguousarray(zc.T).astype(BF16)
        slots = []   # (stage, expert)
        for s in range(3):
            pl = plans[s]
            for j in range(NCH):
                slots.append((s, int(pl["chunk_e"][c][j])))
            for (r, e, w) in pl["assign"][c]:
                slots.append((s, int(e)))
        Wpk = np.empty((S_tot * P, KO * LD), BF16)
        bias = np.empty((P, S_tot * KO), np.float32)
        for i, (s, e) in enumerate(slots):
            Wpk[i * P:(i + 1) * P] = Wpk_s[s][e]
            bias[:, i * KO:(i + 1) * KO] = bs[s][e].reshape(KO, P).T
        desc = np.zeros((1, max(F_tot, 1)), np.int32)
        i = 0
        for s in range(3):
            for (r, e, w) in plans[s]["assign"][c]:
                desc[0, i] = r
                i += 1
        in_maps.append({
            "zT": zTc,
            "Wpk": Wpk,
            "bias": bias,
            "desc": desc,
        })
    return in_maps


def _kernel_numpy_fallback(z, Ws, bs, ids_all):
    out = np.asarray(z, np.float32)
    for s in range(3):
        nxt = np.empty_like(out)
        ids = ids_all[s]
        for e in range(Ws[s].shape[0]):
            mask = ids == e
            if mask.any():
                nxt[mask] = np.maximum(out[mask] @ Ws[s][e] + bs[s][e], 0.0)
        out = nxt
    return out


def kernel(z, W_dataset, b_dataset, W_assay, b_assay, W_donor, b_donor,
           dataset_ids, assay_ids, donor_ids):
    global LAST_RESULTS
    ids_all = [
        np.asarray(dataset_ids, np.int32),
        np.asarray(assay_ids, np.int32),
        np.asarray(donor_ids, np.int32),
    ]
    Ws = [
        np.ascontiguousarray(np.asarray(W_dataset, np.float32)),
        np.ascontiguousarray(np.asarray(W_assay, np.float32)),
        np.ascontiguousarray(np.asarray(W_donor, np.float32)),
    ]
    bs = [
        np.asarray(b_dataset, np.float32),
        np.asarray(b_assay, np.float32),
        np.asarray(b_donor, np.float32),
    ]
    if any(np.any(np.diff(ids) < 0) for ids in ids_all):
        return _kernel_numpy_fallback(z, Ws, bs, ids_all)

    import os
    rows = None
    plans = None
    if os.environ.get("MOE_ASSIGN", "1") == "1":
        for maker in (_construct_rows, _assign_rows):
            try:
                rows = maker(ids_all)
            except AssertionError:
                rows = None
            if rows is None:
                continue
            plans = _make_plans(ids_all, rows)
            if plans is not None and _validate(ids_all, rows, plans):
                break
            rows, plans = None, None
    if plans is None:
        rows = None
        plans = _make_plans(ids_all, rows)
    if plans is None or not _validate(ids_all, rows, plans):
        return _kernel_numpy_fallback(z, Ws, bs, ids_all)
    F = tuple(tuple(zip(plans[s]["slots"], plans[s]["after"]))
              for s in range(3))
    has_bias = any(np.any(b) for b in bs)
    key = (F, has_bias)
    if key not in _program_cache:
        _program_cache[key] = _build_program(F, has_bias)
    nc = _program_cache[key]
    in_maps = _prep_inputs(z, Ws, bs, plans, rows)
    if not has_bias:
        for m in in_maps:
            m.pop("bias", None)
    res = bass_utils.run_bass_kernel_spmd(nc, in_maps,
                                          core_ids=list(range(NCORES)))
    LAST_RESULTS = res

    W2 = plans[2]["slots"]
    W2_off = np.concatenate([[0], np.cumsum(W2)]).astype(int) if W2 \
        else np.zeros(1, int)
    out = np.empty((N, LD), np.float32)
    for c in range(NCORES):
        blk = res.results[c]["outT"].astype(np.float32).T  # [SH, LD] pre-fix
        fo = res.results[c]["fixO"].astype(np.float32).T   # [sum(W2), LD]
        for f in range(len(W2)):
            r, e, w = plans[2]["assign"][c][f]
            if r >= SH:
                continue
            n = min(w, SH - r)
            blk[r:r + n] = fo[W2_off[f]:W2_off[f] + n]
        if rows is not None:
            out[rows[c]] = blk
        else:
            out[c * SH:(c + 1) * SH] = blk
    return out


# revision 64
# speedup vs baseline: 1.1340x; 1.0099x over previous
"""Trainium2 Bass kernel for nn_DFVAE (3-stage MoE routing with sorted ids).

Static chunk-grid strategy (N=16384, LD=512, experts (8,6,16), 8 cores,
bf16 end-to-end), v2:
  - Per (core, stage): 4 STATIC 512-row chunk windows (expert = id at the
    chunk start) plus F_s dynamic fix windows with PER-SLOT STATIC WIDTHS
    (program signature) that rewrite rows between an unaligned run start and
    the next chunk boundary.  Unused fix slots park at row SH (the zeroed
    pad), writing zeros to pad rows only.
  - Weights host-packed PER CORE in window-slot order (bf16 lhsT layout);
    matmul lhsT addresses are static.  Fix windows use values_load row
    offsets from a desc vector.
  - Activations bf16 in two ping-pong SBUF tiles; stage-2 chunk outputs
    stored in k01/k23 halves as soon as their evacs land; stage-2 fixes
    (ordered widest..narrowest) write disjoint fixO staging merged on host.
  - Head: chunk-0 weights+z arrive in k-quarters and chunk 0 runs a k-outer
    loop so the first matmul needs only 2 small DMAs; PE p-state ramp is
    held by warm-up matmuls on a tiny zeroed tile.
"""
import numpy as np
import ml_dtypes

import concourse.mybir as mybir
import concourse.tile as tile
from concourse import bacc, bass_utils
from concourse.bass import ds

N = 16384
LD = 512
NCORES = 8
SH = N // NCORES      # 2048 rows per core
P = 128
KO = LD // P          # 4 contraction/feature subtiles
CH = 512              # static chunk rows
NCH = SH // CH        # 4 chunks per core
PAD = 512             # activation tile pad rows (fix spill + filler park)
STAGE_E = (8, 6, 16)

BF16 = ml_dtypes.bfloat16

LAST_RESULTS = None  # test harness reads exec_time_ns off this

_program_cache = {}

WARMUP = 63
GAPFILL = (0, 0, 0, 0)


def _stage_fixes(loc):
    """Per-breakpoint fix options for one core's id vector.

    Each chunk's expert is the value of its LONGEST run, so every other run
    inside the chunk needs one fix window.  A fix for run [a, b) may be
    anchored at its end (r = b - W, W >= b - a) because back-spill rewrites
    same-run rows of the previous chunk with identical values, or at its
    start when the run crosses into the next chunk (forward spill rewrites
    same-run rows the next chunk also computes).

    Returns (chunk_e[NCH], fixes=[(anchor, expert, need, maxw)...]) where
    anchor is 'r' (r = bp, forward) or 'l' (r = end - W, backward); fixes
    sorted by position.
    """
    starts = np.flatnonzero(np.diff(loc)) + 1
    bounds = np.concatenate([[0], starts, [SH]]).astype(np.int64)
    chunk_e = []
    fixes = []
    for j in range(NCH):
        c0, c1 = j * CH, (j + 1) * CH
        # runs intersecting this chunk, clipped
        runs = []
        for i in range(len(bounds) - 1):
            a, b = int(bounds[i]), int(bounds[i + 1])
            if b <= c0 or a >= c1:
                continue
            runs.append((max(a, c0), min(b, c1), int(loc[max(a, c0)]),
                         a, b))  # clipped start/end, value, full start/end
        best_j = max(range(len(runs)),
                     key=lambda i: runs[i][1] - runs[i][0])
        chunk_e.append(runs[best_j][2])
        for i, (a, b, v, fa, fb) in enumerate(runs):
            if i == best_j:
                continue
            need = b - a
            if fb > c1:
                # run crosses chunk end: forward-anchored, may spill right
                fixes.append(('r', a, v, need, min(fb - a, CH)))
            else:
                # backward-anchored at run (clipped) end, may spill left
                fixes.append(('l', b, v, need, min(b - fa, CH)))
    fixes.sort(key=lambda f: f[1])
    return chunk_e, fixes


def slot_cost(slots):
    # a fix window costs ~2.1us of PE-SEQ dispatch (partially hidden under
    # chunk slack) and 6.67ns/row of PE-engine time
    return sum(max(2100.0, 6.67 * w + 400.0) for w in slots)


def _match_core(fixes, slots):
    """Injective fix->slot matching with need <= W <= maxw; None if none.

    Slots sorted desc; prefer using wider slots for wider needs.  F is tiny
    (<=5) so brute-force permutations are fine.
    """
    import itertools
    F = len(slots)
    n = len(fixes)
    order = sorted(range(n), key=lambda i: -fixes[i][3])
    for perm in itertools.permutations(range(F), n):
        ok = True
        for i, k in zip(order, perm):
            _, _, _, need, maxw = fixes[i]
            if not (need <= slots[k] <= maxw):
                ok = False
                break
        if ok:
            asg = [None] * F
            for i, k in zip(order, perm):
                anchor, pos, v, need, maxw = fixes[i]
                W = slots[k]
                r = pos if anchor == 'r' else pos - W
                asg[k] = (r, v, W)
            for k in range(F):
                if asg[k] is None:
                    asg[k] = (SH, 0, slots[k])
            return asg
    return None


def _plan_stage(per_core):
    """Slot widths for one stage with spill-allowance-aware matching.

    Returns (slots, assign) or None.  assign[c] = [(r, expert, W)...] in
    slot order with fillers (r=SH, W) for unused slots.
    """
    lists = [sorted(fixes, key=lambda f: -f[3]) for _, fixes in per_core]
    F = max((len(fx) for fx in lists), default=0)
    base = [0] * F
    for fx in lists:
        for k, f in enumerate(fx):
            base[k] = max(base[k], f[3])
    # candidate slot multisets: rank-max needs, then exact per-core widths
    cands = [base]
    widths = sorted({f[3] for fx in lists for f in fx}, reverse=True)
    if widths and len(widths) <= F + 2:
        from collections import Counter
        mult = Counter()
        for fx in lists:
            cc = Counter(f[3] for f in fx)
            for w, n in cc.items():
                mult[w] = max(mult[w], n)
        exact = sorted(mult.elements(), reverse=True)
        if len(exact) <= F + 2:
            cands.append(exact)
    for slots in cands:
        assign = []
        ok = True
        for fx in lists:
            asg = _match_core(fx, slots)
            if asg is None:
                ok = False
                break
            assign.append(asg)
        if ok:
            return slots, assign
    return None


def _validate(ids_all, rows, plans):
    """Simulate chunk+fix writes in device window order; True iff ids
    reproduced.  Stages 0/1 write in place: chunk j, then fixes placed
    after chunk j (slot order).  Stage 2 fixes go to staging and the host
    merges them after all chunks, i.e. same ordering semantics."""
    for s in range(3):
        plan = plans[s]
        after = plan["after"]
        for c in range(NCORES):
            loc = ids_all[s][rows[c]] if rows is not None \
                else ids_all[s][c * SH:(c + 1) * SH]
            arr = np.empty(SH + PAD, np.int64)
            arr[:] = -1
            for j in range(NCH):
                arr[j * CH:(j + 1) * CH] = plan["chunk_e"][c][j]
                if s == 2:
                    continue  # stage-2 fixes merge on host after all chunks
                for k, (r, e, w) in enumerate(plan["assign"][c]):
                    if after[k] == j:
                        arr[r:r + w] = e
            if s == 2:
                for (r, e, w) in plan["assign"][c]:
                    arr[r:r + w] = e
            if not np.array_equal(arr[:SH], loc):
                return False
    return True


def _make_plans(ids_all, rows):
    plans = []
    for s in range(3):
        per_core = []
        for c in range(NCORES):
            loc = ids_all[s][rows[c]] if rows is not None \
                else ids_all[s][c * SH:(c + 1) * SH]
            per_core.append(_stage_fixes(loc))
        ps = _plan_stage(per_core)
        if ps is None:
            return None
        slots, assign = ps
        # earliest safe interleave point per slot: after the last chunk any
        # core's matched fix touches (incl. spill).  Stage 0 fixes read z at
        # dynamic offsets so they stay at stage end; stage 2 fixes write
        # disjoint staging merged on host, so they can run immediately.
        F = len(slots)
        after = []
        for k in range(F):
            if s == 0:
                after.append(NCH - 1)
                continue
            if s == 2:
                after.append(min(k, NCH - 2))
                continue
            aj = 0
            for c in range(NCORES):
                r, e, w = assign[c][k]
                if r < SH:
                    aj = max(aj, (r + w - 1) // CH)
            after.append(min(aj, NCH - 1))
        plans.append({
            "slots": slots,
            "chunk_e": [pc[0] for pc in per_core],
            "assign": assign,
            "after": after,
        })
    return plans


def _construct_rows(ids_all):
    """Quantum dealing: 32 contiguous 512-row quanta, grouped so every
    breakpoint's quantum is adjacent (in its core) to the neighbor quantum
    its fix window spills into, then packed into cores balancing fix load.

    Returns rows[8] or None.
    """
    NQ = N // CH
    bps = []
    for s in range(3):
        for bp in (np.flatnonzero(np.diff(ids_all[s])) + 1).tolist():
            bps.append((int(bp), s))
    # union quanta that must stay adjacent (in global order)
    parent = list(range(NQ))

    def find(a):
        while parent[a] != a:
            parent[a] = parent[parent[a]]
            a = parent[a]
        return a

    def union(a, b):
        ra, rb = find(a), find(b)
        if ra != rb:
            parent[max(ra, rb)] = min(ra, rb)

    for pos, s in bps:
        q, x = pos // CH, pos % CH
        if x == 0:
            continue
        if x <= CH // 2 and q >= 1:
            union(q - 1, q)
        elif x > CH // 2 and q + 1 < NQ:
            union(q, q + 1)
    groups = {}
    for q in range(NQ):
        groups.setdefault(find(q), []).append(q)
    pieces = sorted(groups.values(), key=len, reverse=True)
    if pieces and len(pieces[0]) > NCH:
        return None
    # fix load per piece, per stage: sorted-desc width lists
    def piece_load(p):
        lo, hi = p[0] * CH, (p[-1] + 1) * CH
        load = [[], [], []]
        for pos, s in bps:
            if lo <= pos < hi and pos % CH:
                x = pos % CH
                load[s].append(min(x, CH - x))
        for s in range(3):
            load[s].sort(reverse=True)
        return load

    loads = [piece_load(p) for p in pieces]
    cores = [[] for _ in range(NCORES)]
    core_load = [[[], [], []] for _ in range(NCORES)]

    def inc_cost(cl, pl):
        inc = 0.0
        tgt = (1, 1, 2)  # window-count targets per stage
        for s in range(3):
            cur = sorted(cl[s], reverse=True)
            new = sorted(cl[s] + pl[s], reverse=True)
            inc += (sum(max(2100.0, 6.67 * w) for w in new)
                    - sum(max(2100.0, 6.67 * w) for w in cur))
            inc += 4000.0 * max(0, len(new) - tgt[s])
        return inc

    order = sorted(range(len(pieces)),
                   key=lambda i: (-len(pieces[i]),
                                  -sum(sum(l) for l in loads[i])))
    for i in order:
        best_c, best = None, None
        for c in range(NCORES):
            if sum(len(p) for p in cores[c]) + len(pieces[i]) > NCH:
                continue
            v = inc_cost(core_load[c], loads[i])
            if best is None or v < best:
                best_c, best = c, v
        if best_c is None:
            return None
        cores[best_c].append(pieces[i])
        for s in range(3):
            core_load[best_c][s].extend(loads[i][s])
    rows = []
    for c in range(NCORES):
        # boundary-carrying pieces first: their fix windows then sit in low
        # chunks and can interleave early between chunk windows
        def piece_fixes(p):
            lo, hi = p[0] * CH, (p[-1] + 1) * CH
            return -sum(1 for pos, s in bps if lo <= pos < hi and pos % CH)
        idx = []
        for p in sorted(cores[c], key=piece_fixes):
            idx.append(np.arange(p[0] * CH, (p[-1] + 1) * CH,
                                 dtype=np.int64))
        rows.append(np.concatenate(idx))
    return rows


def _assign_rows(ids_all):
    """Order-preserving row->core assignment that steers breakpoint offsets.

    Beam over per-core donations: core c may defer a tail slice of an atomic
    block to core c+1.  Scores by total fix width (exact widths) with
    per-stage max-across-cores structure.
    """
    trip = ids_all[0].astype(np.int64) * 10000 + ids_all[1] * 100 + ids_all[2]
    block_of = np.cumsum(np.diff(trip, prepend=trip[0]) != 0)

    def fix_needs(loc):
        """Sorted-desc fix widths for one core/stage."""
        _, fixes = _stage_fixes(loc)
        return sorted((f[3] for f in fixes), reverse=True)

    def merge_slots(slots, needs):
        out = list(slots) + [0] * max(0, len(needs) - len(slots))
        for k, w in enumerate(needs):
            out[k] = max(out[k], w)
        return out

    beam = [(0, 0, np.empty(0, np.int64), [[], [], []], [])]
    for c in range(NCORES):
        nxt_states = []
        for _, cursor, bag, fmax, rs in beam:
            b = len(bag)
            cands = {(0, 0)}
            if c < NCORES - 1:
                base_end = cursor + SH - b
                blks = block_of[cursor:base_end]
                ends = np.flatnonzero(np.diff(blks)) + 1
                los = np.concatenate([[0], ends])
                his = np.concatenate([ends, [SH - b]])
                loc0 = np.concatenate([bag, np.arange(cursor, base_end)])
                wants = set()
                for s in range(3):
                    loc = ids_all[s][loc0]
                    for bp in (np.flatnonzero(np.diff(loc)) + 1):
                        off = int(bp) % CH
                        for tgt in (0, 128, 192, 256, 320, 384, 416, 448,
                                    480, 496):
                            gg = (tgt - off) % CH
                            if gg:
                                wants.add((int(bp), gg))
                for lo, hi in zip(los.tolist(), his.tolist()):
                    avail = hi - lo
                    glo = cursor + hi
                    for bp, gg in wants:
                        if gg < avail and b + hi <= bp + gg:
                            cands.add((glo, gg))
                    for gfix in (128, 256, 384):
                        if avail > gfix:
                            cands.add((glo, gfix))
            for hi, g in sorted(cands):
                take = SH - b + g
                if cursor + take > N or (c == NCORES - 1 and
                                         cursor + take != N):
                    continue
                if g:
                    idx = np.concatenate([
                        bag,
                        np.arange(cursor, hi - g),
                        np.arange(hi, cursor + take),
                    ])
                else:
                    idx = np.concatenate([bag,
                                          np.arange(cursor, cursor + take)])
                if len(idx) != SH:
                    continue
                nf = [merge_slots(fmax[s], fix_needs(ids_all[s][idx]))
                      for s in range(3)]
                key = (sum(slot_cost(x) for x in nf), )
                nbag = np.arange(hi - g, hi) if g else np.empty(0, np.int64)
                nxt_states.append((key, cursor + take, nbag, nf, rs + [idx]))
        nxt_states.sort(key=lambda st: st[0])
        seen = set()
        beam = []
        for st in nxt_states:
            sk = (st[1], len(st[2]), int(st[2][0]) if len(st[2]) else -1)
            if sk in seen:
                continue
            seen.add(sk)
            beam.append(st)
            if len(beam) >= 48:
                break
    key, cursor, bag, fmax, rows = beam[0]
    assert cursor == N and len(bag) == 0, (cursor, len(bag))
    return rows


def _pack_w(W):
    """[E, LD, LD] -> [E, P, KO*LD] lhsT layout (k-major blocks)."""
    E = W.shape[0]
    return np.ascontiguousarray(
        W.reshape(E, KO, P, LD).transpose(0, 2, 1, 3).reshape(E, P, KO * LD))


def _build_program(F, has_bias=True):
    # F[s] = tuple of (width, after_chunk) per fix slot
    AF = [[a for _, a in F[s]] for s in range(3)]
    W0, W1, W2 = [tuple(w for w, _ in F[s]) for s in range(3)]
    Fs = [len(W0), len(W1), len(W2)]
    S = [NCH + Fs[s] for s in range(3)]
    S_tot = sum(S)
    F_tot = sum(Fs)
    FMAX = max([64] + list(W0) + list(W1) + list(W2))
    W2_off = np.concatenate([[0], np.cumsum(W2)]).astype(int) if Fs[2] \
        else np.zeros(1, int)
    W2_tot = int(W2_off[-1]) if Fs[2] else 0

    nc = bacc.Bacc("TRN2", target_bir_lowering=False, debug=False,
                   enable_asserts=False, num_devices=NCORES)
    bf = mybir.dt.bfloat16
    f32 = mybir.dt.float32
    i32 = mybir.dt.int32
    PE = mybir.EngineType.PE
    ACT = mybir.EngineType.Activation
    DVE = mybir.EngineType.DVE

    ND = max(F_tot, 1)
    zT = nc.dram_tensor("zT", [LD, SH], bf, kind="ExternalInput").ap()
    Wt = nc.dram_tensor("Wpk", [S_tot * P, KO * LD], bf,
                        kind="ExternalInput").ap()
    Bt = (nc.dram_tensor("bias", [P, S_tot * KO], f32,
                         kind="ExternalInput").ap() if has_bias else None)
    Dt = nc.dram_tensor("desc", [1, ND], i32, kind="ExternalInput").ap()
    Ot = nc.dram_tensor("outT", [LD, SH], bf, kind="ExternalOutput").ap()
    Ft = nc.dram_tensor("fixO", [LD, max(W2_tot, 64)], bf,
                        kind="ExternalOutput").ap()

    zv = zT.rearrange("(ko p) r -> p ko r", p=P)
    ov = Ot.rearrange("(ko p) r -> p ko r", p=P)
    fv = Ft.rearrange("(ko p) r -> p ko r", p=P)
    Wv = Wt.rearrange("(s p) c -> s p c", p=P)

    soff = [0, S[0], S[0] + S[1]]
    doff = [0, Fs[0], Fs[0] + Fs[1]]
    H = KO * LD // 2   # half of a weight slot's columns
    Q = KO * LD // 4   # quarter (one k block)

    with tile.TileContext(nc) as tc:
        with (
            tc.tile_pool(name="const", bufs=1) as cpool,
            tc.tile_pool(name="ps512", bufs=5, space="PSUM") as pp5,
            tc.tile_pool(name="psfix", bufs=3, space="PSUM") as ppf,
        ):
            actA = cpool.tile([P, KO, SH + PAD], bf)
            actB = cpool.tile([P, KO, SH + PAD], bf)
            zdum = cpool.tile([P, P + 64], bf)
            fixout = [cpool.tile([P, KO, W2[f]], bf, name=f"fo{f}",
                                 tag=f"fo{f}") for f in range(Fs[2])]

            w_sb = []
            for s in range(3):
                row = [cpool.tile([P, KO * LD], bf, name=f"w{s}_{j}",
                                  tag=f"w{s}_{j}") for j in range(S[s])]
                w_sb.append(row)

            # zdum first so warm-up matmuls start as early as possible
            nc.gpsimd.memset(zdum[:], 0.0)
            nc.gpsimd.memset(actA[:, :, SH:SH + PAD], 0.0)
            nc.gpsimd.memset(actB[:, :, SH:SH + PAD], 0.0)

            # head: chunk-0 weights+z arrive as per-k quarter pairs so the
            # k-outer chunk-0 loop is never starved; chunk 1 in k01/k23
            # halves; chunks 2-3 whole; desc + stage-0 fix weights early so
            # interleaved fix windows aren't blocked
            for j in range(2):
                for h in range(2):
                    nc.sync.dma_start(
                        w_sb[0][j][:, h * H:(h + 1) * H],
                        Wv[soff[0] + j][:, h * H:(h + 1) * H])
                    nc.sync.dma_start(
                        actA[:, 2 * h:2 * h + 2, j * CH:(j + 1) * CH],
                        zv[:, 2 * h:2 * h + 2, j * CH:(j + 1) * CH])
            if has_bias:
                bias_sb = cpool.tile([P, S_tot * KO], f32)
                nc.sync.dma_start(bias_sb[:, 0:NCH * KO], Bt[:, 0:NCH * KO])
            for j in range(2, NCH):
                nc.sync.dma_start(w_sb[0][j][:], Wv[soff[0] + j])
                nc.sync.dma_start(actA[:, :, j * CH:(j + 1) * CH],
                                  zv[:, :, j * CH:(j + 1) * CH])
            if has_bias:
                nc.sync.dma_start(bias_sb[:, NCH * KO:], Bt[:, NCH * KO:])
            desc_sb = cpool.tile([1, ND], i32)
            nc.sync.dma_start(desc_sb[:], Dt)
            for f in range(Fs[0]):
                nc.sync.dma_start(w_sb[0][NCH + f][:], Wv[soff[0] + NCH + f])
            for s in (1, 2):
                for j in range(S[s]):
                    nc.sync.dma_start(w_sb[s][j][:], Wv[soff[s] + j])

            # PE p-state warm-up on the small zeroed tile
            psw = ppf.tile([P, FMAX], f32, tag="psf", name="psw")

            def dummies(n):
                for i in range(n):
                    nc.tensor.matmul(psw[:, 0:64],
                                     lhsT=zdum[:, 0:P],
                                     rhs=zdum[:, P:P + 64],
                                     start=True, stop=True)

            dummies(WARMUP)

            def evac(m, dst_ap, psum_ap, bias_ap):
                if m % 2 == 0:
                    if has_bias:
                        nc.scalar.activation(dst_ap, psum_ap,
                                             mybir.ActivationFunctionType.Relu,
                                             bias=bias_ap)
                    else:
                        nc.scalar.activation(dst_ap, psum_ap,
                                             mybir.ActivationFunctionType.Relu)
                else:
                    nc.vector.tensor_scalar(dst_ap, psum_ap,
                                            bias_ap if has_bias else 0.0, 0.0,
                                            mybir.AluOpType.add,
                                            mybir.AluOpType.max)

            def bias_ap_of(s, slot, m):
                if not has_bias:
                    return None
                col = (soff[s] + slot) * KO + m
                return bias_sb[:, col:col + 1]

            def chunk_koru(s, cur, nxt, j):
                """k-outer chunk: k01 matmuls start as soon as the first
                half-DMA pair lands."""
                wt = w_sb[s][j]
                lo = j * CH
                psums = [pp5.tile([P, CH], f32, tag="ps", name="psum")
                         for _ in range(KO)]
                for k in range(KO):
                    for m in range(KO):
                        nc.tensor.matmul(
                            psums[m][:],
                            lhsT=wt[:, k * LD + m * P: k * LD + (m + 1) * P],
                            rhs=cur[:, k, lo:lo + CH],
                            start=(k == 0), stop=(k == KO - 1))
                for m in range(KO):
                    evac(m, nxt[:, m, lo:lo + CH], psums[m][:],
                         bias_ap_of(s, j, m))

            def chunk_window(s, cur, nxt, j):
                wt = w_sb[s][j]
                lo = j * CH
                for m in range(KO):
                    psum = pp5.tile([P, CH], f32, tag="ps", name="psum")
                    for k in range(KO):
                        nc.tensor.matmul(
                            psum[:],
                            lhsT=wt[:, k * LD + m * P: k * LD + (m + 1) * P],
                            rhs=cur[:, k, lo:lo + CH],
                            start=(k == 0), stop=(k == KO - 1))
                    evac(m, nxt[:, m, lo:lo + CH], psum[:],
                         bias_ap_of(s, j, m))
                    if s == 2 and m == 1:
                        nc.sync.dma_start(ov[:, 0:2, lo:lo + CH],
                                          nxt[:, 0:2, lo:lo + CH])
                    elif s == 2 and m == KO - 1:
                        nc.sync.dma_start(ov[:, 2:4, lo:lo + CH],
                                          nxt[:, 2:4, lo:lo + CH])

            def chunk_tail(s, cur, nxt):
                """Stage-2 chunk 3 as 256+128+128 sub-windows: each
                sub-window's store chain clears HWDGE before the next one's
                evac lands, so the kernel-final chain starts from a 128-row
                evac with an idle issue path."""
                j = NCH - 1
                wt = w_sb[s][j]
                base = j * CH
                for (off, w) in ((0, CH - 64), (CH - 64, 64)):
                    lo = base + off
                    for m in range(KO):
                        psum = pp5.tile([P, CH], f32, tag="ps", name="psum")
                        for k in range(KO):
                            nc.tensor.matmul(
                                psum[:, 0:w],
                                lhsT=wt[:, k * LD + m * P:
                                        k * LD + (m + 1) * P],
                                rhs=cur[:, k, lo:lo + w],
                                start=(k == 0), stop=(k == KO - 1))
                        evac(m, nxt[:, m, lo:lo + w], psum[:, 0:w],
                             bias_ap_of(s, j, m))
                        if off == 0 and m == 1:
                            nc.sync.dma_start(ov[:, 0:2, lo:lo + w],
                                              nxt[:, 0:2, lo:lo + w])
                        elif off == 0 and m == KO - 1:
                            nc.sync.dma_start(ov[:, 2:4, lo:lo + w],
                                              nxt[:, 2:4, lo:lo + w])
                        elif off and m == KO - 1:
                            nc.sync.dma_start(ov[:, :, lo:lo + w],
                                              nxt[:, :, lo:lo + w])

            def fix_window(s, cur, nxt, f, W):
                idx = doff[s] + f
                last = s == 2
                r = nc.values_load(
                    desc_sb[0:1, idx:idx + 1],
                    engines=[PE] if last else [PE, ACT, DVE],
                    min_val=0, max_val=SH,
                    skip_runtime_bounds_check=True)
                wt = w_sb[s][NCH + f]
                fo = fixout[f] if last else None
                for m in range(KO):
                    psum = ppf.tile([P, FMAX], f32, tag="psf", name="psumf")
                    for k in range(KO):
                        nc.tensor.matmul(
                            psum[:, 0:W],
                            lhsT=wt[:, k * LD + m * P: k * LD + (m + 1) * P],
                            rhs=cur[:, k, ds(r, W)],
                            start=(k == 0), stop=(k == KO - 1))
                    bap = bias_ap_of(s, NCH + f, m)
                    if last:
                        evac(m, fo[:, m, :], psum[:, 0:W], bap)
                        if m == 1:
                            nc.sync.dma_start(
                                fv[:, 0:2, W2_off[f]:W2_off[f + 1]],
                                fo[:, 0:2, :])
                        elif m == KO - 1:
                            nc.sync.dma_start(
                                fv[:, 2:4, W2_off[f]:W2_off[f + 1]],
                                fo[:, 2:4, :])
                    else:
                        evac(m, nxt[:, m, ds(r, W)], psum[:, 0:W], bap)

            for s in range(3):
                cur, nxt = (actA, actB) if s % 2 == 0 else (actB, actA)
                Ws = (W0, W1, W2)[s]
                for j in range(NCH):
                    if s == 0 and j < 2:
                        chunk_koru(s, cur, nxt, j)
                        dummies(GAPFILL[j])
                    elif s == 2 and j == NCH - 1:
                        chunk_tail(s, cur, nxt)
                    else:
                        chunk_window(s, cur, nxt, j)
                        if s == 0:
                            dummies(GAPFILL[j])
                    # fix slot k runs right after the last chunk any core's
                    # matched fix touches (stage-0 fixes pinned to stage end)
                    for f in range(Fs[s]):
                        if AF[s][f] == j:
                            fix_window(s, cur, nxt, f, Ws[f])
    nc.compile()
    return nc


def _prep_inputs(z, Ws, bs, plans, rows):
    Fs = [len(plans[s]["slots"]) for s in range(3)]
    S = [NCH + f for f in Fs]
    S_tot = sum(S)
    F_tot = sum(Fs)
    Wpk_s = [_pack_w(Ws[s]).astype(BF16) for s in range(3)]
    z32 = np.asarray(z, np.float32)

    in_maps = []
    for c in range(NCORES):
        zc = z32[rows[c]] if rows is not None else z32[c * SH:(c + 1) * SH]
        zTc = np.ascontiguousarray(zc.T).astype(BF16)
        slots = []   # (stage, expert)
        for s in range(3):
            pl = plans[s]
            for j in range(NCH):
                slots.append((s, int(pl["chunk_e"][c][j])))
            for (r, e, w) in pl["assign"][c]:
                slots.append((s, int(e)))
        Wpk = np.empty((S_tot * P, KO * LD), BF16)
        bias = np.empty((P, S_tot * KO), np.float32)
        for i, (s, e) in enumerate(slots):
            Wpk[i * P:(i + 1) * P] = Wpk_s[s][e]
            bias[:, i * KO:(i + 1) * KO] = bs[s][e].reshape(KO, P).T
        desc = np.zeros((1, max(F_tot, 1)), np.int32)
        i = 0
        for s in range(3):
            for (r, e, w) in plans[s]["assign"][c]:
                desc[0, i] = r
                i += 1
        in_maps.append({
            "zT": zTc,
            "Wpk": Wpk,
            "bias": bias,
            "desc": desc,
        })
    return in_maps


def _kernel_numpy_fallback(z, Ws, bs, ids_all):
    out = np.asarray(z, np.float32)
    for s in range(3):
        nxt = np.empty_like(out)
        ids = ids_all[s]
        for e in range(Ws[s].shape[0]):
            mask = ids == e
            if mask.any():
                nxt[mask] = np.maximum(out[mask] @ Ws[s][e] + bs[s][e], 0.0)
        out = nxt
    return out


def kernel(z, W_dataset, b_dataset, W_assay, b_assay, W_donor, b_donor,
           dataset_ids, assay_ids, donor_ids):
    global LAST_RESULTS
    ids_all = [
        np.asarray(dataset_ids, np.int32),
        np.asarray(assay_ids, np.int32),
        np.asarray(donor_ids, np.int32),
    ]
    Ws = [
        np.ascontiguousarray(np.asarray(W_dataset, np.float32)),
        np.ascontiguousarray(np.asarray(W_assay, np.float32)),
        np.ascontiguousarray(np.asarray(W_donor, np.float32)),
    ]
    bs = [
        np.asarray(b_dataset, np.float32),
        np.asarray(b_assay, np.float32),
        np.asarray(b_donor, np.float32),
    ]
    if any(np.any(np.diff(ids) < 0) for ids in ids_all):
        return _kernel_numpy_fallback(z, Ws, bs, ids_all)

    import os
    rows = None
    plans = None
    if os.environ.get("MOE_ASSIGN", "1") == "1":
        for maker in (_construct_rows, _assign_rows):
            try:
                rows = maker(ids_all)
            except AssertionError:
                rows = None
            if rows is None:
                continue
            plans = _make_plans(ids_all, rows)
            if plans is not None and _validate(ids_all, rows, plans):
                break
            rows, plans = None, None
    if plans is None:
        rows = None
        plans = _make_plans(ids_all, rows)
    if plans is None or not _validate(ids_all, rows, plans):
        return _kernel_numpy_fallback(z, Ws, bs, ids_all)
    F = tuple(tuple(zip(plans[s]["slots"], plans[s]["after"]))
              for s in range(3))
    has_bias = any(np.any(b) for b in bs)
    key = (F, has_bias)
    if key not in _program_cache:
        _program_cache[key] = _build_program(F, has_bias)
    nc = _program_cache[key]
    in_maps = _prep_inputs(z, Ws, bs, plans, rows)
    if not has_bias:
        for m in in_maps:
            m.pop("bias", None)
    res = bass_utils.run_bass_kernel_spmd(nc, in_maps,
                                          core_ids=list(range(NCORES)))
    LAST_RESULTS = res

    W2 = plans[2]["slots"]
    W2_off = np.concatenate([[0], np.cumsum(W2)]).astype(int) if W2 \
        else np.zeros(1, int)
    out = np.empty((N, LD), np.float32)
    for c in range(NCORES):
        blk = res.results[c]["outT"].astype(np.float32).T  # [SH, LD] pre-fix
        fo = res.results[c]["fixO"].astype(np.float32).T   # [sum(W2), LD]
        for f in range(len(W2)):
            r, e, w = plans[2]["assign"][c][f]
            if r >= SH:
                continue
            n = min(w, SH - r)
            blk[r:r + n] = fo[W2_off[f]:W2_off[f] + n]
        if rows is not None:
            out[rows[c]] = blk
        else:
            out[c * SH:(c + 1) * SH] = blk
    return out
